# revision 2
# baseline (speedup 1.0000x reference)
"""Spiking ConvNet (LIF) Bass kernel for nn_ConvNet_70720931496461 on 8 TRN2 cores.

Data-parallel over batch (B=256 -> 8 x 32). Per core, per timestep:
  conv1 (kn2row weight-stationary GEMM) -> LIF0 (fused DVE stt ops) -> maxpool
  -> conv2 (weight-stationary, col-packed A/B PSUM) -> PE-transpose to
  pixel-major -> LIF1 -> pool -> PE-transpose -> FC GEMM -> LIF2 -> out LI.

i-states are scaled by 0.1 (injection scale folded into weights host-side).
All matmuls bf16; states bf16; PSUM f32.

Layout L0: partition p = jq*64 + jp*20 + oc  (j = 2*jp + jq, r = rg*6 + j),
free = (b:32, rg:4, c:24) = 3072.
Layout L1: partition p = qr*64 + qc*32 + b (pixel-parity), free = (r4, c4, oc50).
"""
import numpy as np
import ml_dtypes

import concourse.bass as bass
import concourse.mybir as mybir
import concourse.tile as tile
import concourse.bacc as bacc

BF = ml_dtypes.bfloat16
f32 = mybir.dt.float32
bf16 = mybir.dt.bfloat16
OP = mybir.AluOpType

T, B, ND, BL = 64, 256, 8, 32


def build_nc():
    nc = bacc.Bacc(None, target_bir_lowering=False)

    xrep_d = nc.dram_tensor("xrep", [T, 51, 3072], bf16, kind="ExternalInput")
    w1kn_d = nc.dram_tensor("w1kn", [51, 128], bf16, kind="ExternalInput")
    w2dr_d = nc.dram_tensor("w2dr", [5, 101, 50], bf16, kind="ExternalInput")
    wfct_d = nc.dram_tensor("wfct", [832, 500], bf16, kind="ExternalInput")
    woutt_d = nc.dram_tensor("woutt", [512, 10], bf16, kind="ExternalInput")
    ident_d = nc.dram_tensor("ident", [128, 128], bf16, kind="ExternalInput")
    out_d = nc.dram_tensor("out", [32, 660], f32, kind="ExternalOutput")

    with tile.TileContext(nc) as tc:
        with (
            tc.tile_pool(name="const", bufs=1) as cp,
            tc.tile_pool(name="state", bufs=1) as sp,
            tc.tile_pool(name="xr", bufs=3) as xp,
            tc.tile_pool(name="cbuf", bufs=2) as cbp,
            tc.tile_pool(name="zrp", bufs=2) as zrp,
            tc.tile_pool(name="sa", bufs=2) as sap,
            tc.tile_pool(name="pc", bufs=2, space=bass.MemorySpace.PSUM) as ppc,
            tc.tile_pool(name="pab", bufs=1, space=bass.MemorySpace.PSUM) as pab,
            tc.tile_pool(name="pt", bufs=1, space=bass.MemorySpace.PSUM) as ppt,
            tc.tile_pool(name="pfz", bufs=1, space=bass.MemorySpace.PSUM) as pfz,
        ):
            # ---- constants
            w1sb = cp.tile([51, 128], bf16)
            w2sb = cp.tile([101, 250], bf16)
            wfcsb = cp.tile([128, 3500], bf16)
            woutsb = cp.tile([128, 40], bf16)
            ident = cp.tile([128, 128], bf16)
            nc.sync.dma_start(w1sb[:], w1kn_d[:])
            for dr in range(5):
                nc.sync.dma_start(w2sb[:, dr * 50:(dr + 1) * 50], w2dr_d[dr])
            for k in range(6):
                nc.sync.dma_start(wfcsb[:, k * 500:(k + 1) * 500],
                                  wfct_d[k * 128:(k + 1) * 128, :])
            nc.sync.dma_start(wfcsb[0:64, 3000:3500], wfct_d[768:832, :])
            for k in range(4):
                nc.sync.dma_start(woutsb[:, k * 10:(k + 1) * 10],
                                  woutt_d[k * 128:(k + 1) * 128, :])
            nc.sync.dma_start(ident[:], ident_d[:])

            # ---- states / persistent scratch
            v0 = sp.tile([128, 3072], bf16)
            i0 = sp.tile([128, 3072], bf16)
            vt0 = sp.tile([128, 3072], bf16)
            m1 = sp.tile([128, 1536], bf16)
            zp0 = sp.tile([128, 1540], bf16)
            v1 = sp.tile([128, 800], bf16)
            i1 = sp.tile([128, 800], bf16)
            vt1 = sp.tile([128, 800], bf16)
            n1 = sp.tile([64, 800], bf16)
            v2 = sp.tile([32, 500], bf16)
            i2 = sp.tile([32, 500], bf16)
            vt2 = sp.tile([32, 500], bf16)
            zp1 = sp.tile([32, 832], bf16)
            z2 = sp.tile([32, 512], bf16)
            zT = sp.tile([128, 224], bf16)
            z2T = sp.tile([128, 128], bf16)
            io = sp.tile([32, 10], bf16)
            vob = sp.tile([32, 660], f32)

            for tl in (v0, i0, v1, i1, v2, i2, io):
                nc.gpsimd.memset(tl[:], 0.0)
            nc.gpsimd.memset(vob[:, 0:10], 0.0)
            nc.gpsimd.memset(zp1[:, 800:832], 1.0)
            nc.gpsimd.memset(z2[:, 500:512], 0.0)

            zrep = [[zrp.tile([101, 1536], bf16, tag=f"zr{jp}") for jp in range(3)]
                    for _ in range(2)]
            for pp in range(2):
                for jp in range(3):
                    nc.gpsimd.memset(zrep[pp][jp][100:101, :], 1.0)

            for t in range(T):
                # ================= layer 0 =================
                # v_dec = 0.9*v0 + i0 ; v0 = (v_dec<=1)*v_dec
                nc.vector.scalar_tensor_tensor(
                    vt0[:], v0[:], 0.9, i0[:], OP.mult, OP.add)
                nc.vector.scalar_tensor_tensor(
                    v0[:], vt0[:], 1.0, vt0[:], OP.is_le, OP.mult)
                # pool: c-pairs (free innermost), then jq (partition +-64)
                nc.vector.tensor_reduce(
                    m1[:], vt0[:].rearrange("p (q two) -> p q two", two=2),
                    mybir.AxisListType.X, OP.max)
                nc.vector.tensor_tensor(
                    m1[0:60, :], m1[0:60, :], m1[64:124, :], OP.max)
                # threshold -> pooled spikes (GPS)
                nc.gpsimd.tensor_scalar(
                    zp0[0:60, 0:1536], m1[0:60, :], 1.0, None, OP.is_gt)
                # zrep: 5 dc-shifted copies per jp (pure-shift SBUF DMAs)
                zr = zrep[t % 2]
                for jp in range(3):
                    eng = (nc.sync, nc.scalar, nc.vector)[jp]
                    for dc in range(5):
                        eng.dma_start(
                            zr[jp][dc * 20:(dc + 1) * 20, 0:1536],
                            zp0[jp * 20:(jp + 1) * 20, dc:dc + 1536])

                # conv1: 6 x N=512 weight-stationary MMs; GPS moves psum->c
                xr = xp.tile([51, 3072], bf16, tag="xr")
                for q in range(4):
                    lo = (51 * q) // 4
                    hi = (51 * (q + 1)) // 4
                    nc.sync.dma_start(xr[lo:hi, :], xrep_d[t, lo:hi, :])
                c = cbp.tile([128, 3072], bf16, tag="c")
                for s in range(6):
                    pc = ppc.tile([128, 512], f32, tag="pc")
                    nc.tensor.matmul(
                        pc[:], w1sb[:], xr[:, s * 512:(s + 1) * 512],
                        start=True, stop=True)
                    nc.gpsimd.tensor_copy(c[:, s * 512:(s + 1) * 512], pc[:])
                # i0 = 0.8*i0 + c
                nc.vector.scalar_tensor_tensor(
                    i0[:], i0[:], 0.8, c[:], OP.mult, OP.add)

                # ================= layer 1 =================
                # v-path first (uses i1 from t-1)
                nc.vector.scalar_tensor_tensor(
                    vt1[:], v1[:], 0.9, i1[:], OP.mult, OP.add)
                nc.vector.scalar_tensor_tensor(
                    v1[:], vt1[:], 1.0, vt1[:], OP.is_le, OP.mult)
                nc.vector.tensor_tensor(
                    n1[:], vt1[0:64, :], vt1[64:128, :], OP.max)
                nc.vector.tensor_tensor(
                    n1[0:32, :], n1[0:32, :], n1[32:64, :], OP.max)
                nc.gpsimd.tensor_scalar(
                    zp1[:, 0:800], n1[0:32, :], 1.0, None, OP.is_gt)

                # conv2: 2 b-halves x 5 dr x 3 ro-groups; A (dr even) base 0,
                # B (dr odd) base 64 -> col-packed
                sa = sap.tile([128, 2048], bf16, tag="sa")
                for bh in range(2):
                    pa = pab.tile([128, 1024], f32, tag="pa")
                    pb = pab.tile([128, 1024], f32, tag="pb")
                    mms = []
                    for dr in range(5):
                        for g0 in range(3):
                            ros = [ro for ro in range(8) if (ro + dr) % 3 == g0]
                            mms.append((dr, g0, ros))
                    na = sum(1 for dr, _, _ in mms if dr % 2 == 0)
                    nb = len(mms) - na
                    ia = ib = 0
                    for dr, g0, ros in mms:
                        jp = (ros[0] + dr) % 3
                        rg0 = (ros[0] + dr) // 3
                        ng = len(ros)
                        rhs = (zr[jp][0:101, 0:1536]
                               .rearrange("p (b rg c) -> p b rg c", b=32, rg=4)
                               [:, bh * 16:(bh + 1) * 16, rg0:rg0 + ng, 0:8])
                        if dr % 2 == 0:
                            out = (pa[0:50, :]
                                   .rearrange("p (b ro co) -> p b ro co", b=16, ro=8)
                                   [:, :, ros[0]::3, :][:, :, 0:ng, :])
                            nc.tensor.matmul(out, w2sb[:, dr * 50:(dr + 1) * 50],
                                             rhs, start=(ia == 0), stop=(ia == na - 1))
                            ia += 1
                        else:
                            out = (pb[64:114, :]
                                   .rearrange("p (b ro co) -> p b ro co", b=16, ro=8)
                                   [:, :, ros[0]::3, :][:, :, 0:ng, :])
                            nc.tensor.matmul(out, w2sb[:, dr * 50:(dr + 1) * 50],
                                             rhs, start=(ib == 0), stop=(ib == nb - 1))
                            ib += 1
                    nc.scalar.copy(sa[0:50, bh * 1024:(bh + 1) * 1024], pa[0:50, :])
                    nc.scalar.copy(sa[64:114, bh * 1024:(bh + 1) * 1024],
                                   pb[64:114, :])

                # T2: transpose to pixel-major [128=(qr,qc,b), 800=(rc,oc)], A+B summed
                sa6 = sa[:].rearrange(
                    "p (b rp qr cp qc) -> p rp cp qr qc b", rp=4, qr=2, cp=4, qc=2)
                for half in range(2):
                    pt = ppt.tile([128, 400], f32, tag="pt")
                    for k in range(8):
                        idx = half * 8 + k
                        rp, cpx = idx // 4, idx % 4
                        nc.tensor.matmul(
                            pt[:, k * 50:(k + 1) * 50],
                            sa6[0:50, rp, cpx, :, :, :], ident[0:50, 0:50],
                            is_transpose=True, start=True, stop=False)
                        nc.tensor.matmul(
                            pt[:, k * 50:(k + 1) * 50],
                            sa6[64:114, rp, cpx, :, :, :], ident[64:114, 0:50],
                            is_transpose=True, start=False, stop=True)
                    # i1 += : handled after both halves via two stt ops on slices
                    nc.vector.scalar_tensor_tensor(
                        i1[:, half * 400:(half + 1) * 400],
                        i1[:, half * 400:(half + 1) * 400], 0.8, pt[:],
                        OP.mult, OP.add)

                # ================= layer 2 (FC) =================
                nc.vector.scalar_tensor_tensor(
                    vt2[:], v2[:], 0.9, i2[:], OP.mult, OP.add)
                nc.vector.scalar_tensor_tensor(
                    v2[:], vt2[:], 1.0, vt2[:], OP.is_le, OP.mult)
                nc.gpsimd.tensor_scalar(
                    z2[:, 0:500], vt2[:], 1.0, None, OP.is_gt)

                ptf = pfz.tile([128, 224], f32, tag="fz")
                for k in range(6):
                    nc.tensor.matmul(
                        ptf[:, k * 32:(k + 1) * 32], zp1[:, k * 128:(k + 1) * 128],
                        ident[0:32, 0:32], is_transpose=True)
                nc.tensor.matmul(
                    ptf[0:64, 192:224], zp1[:, 768:832], ident[0:32, 0:32],
                    is_transpose=True)
                nc.scalar.copy(zT[:], ptf[:])

                pf = pfz.tile([32, 500], f32, tag="fz2")
                for k in range(6):
                    nc.tensor.matmul(
                        pf[:], zT[:, k * 32:(k + 1) * 32],
                        wfcsb[:, k * 500:(k + 1) * 500],
                        start=(k == 0), stop=False)
                nc.tensor.matmul(
                    pf[:], zT[0:64, 192:224], wfcsb[0:64, 3000:3500],
                    start=False, stop=True)
                nc.vector.scalar_tensor_tensor(
                    i2[:], i2[:], 0.8, pf[:], OP.mult, OP.add)

                # ================= output LI =================
                pto = pfz.tile([128, 128], f32, tag="fz3")
                for k in range(4):
                    nc.tensor.matmul(
                        pto[:, k * 32:(k + 1) * 32], z2[:, k * 128:(k + 1) * 128],
                        ident[0:32, 0:32], is_transpose=True)
                nc.scalar.copy(z2T[:], pto[:])
                pz = pfz.tile([32, 10], f32, tag="fz4")
                for k in range(4):
                    nc.tensor.matmul(
                        pz[:], z2T[:, k * 32:(k + 1) * 32],
                        woutsb[:, k * 10:(k + 1) * 10],
                        start=(k == 0), stop=(k == 3))
                nc.vector.scalar_tensor_tensor(
                    vob[:, (t + 1) * 10:(t + 2) * 10],
                    vob[:, t * 10:(t + 1) * 10], 0.9, io[:], OP.mult, OP.add)
                nc.vector.scalar_tensor_tensor(
                    io[:], io[:], 0.8, pz[:], OP.mult, OP.add)

            nc.sync.dma_start(out_d[:, :], vob[:, :])

    nc.compile()
    return nc


def host_prep(x, w1, b1, w2, b2, w_fc, b_fc, w_out):
    """Build per-core xrep + shared transformed weights (numpy)."""
    x = np.asarray(x, np.float32).reshape(T, B, 28, 28)
    w1 = np.asarray(w1, np.float32)
    b1 = np.asarray(b1, np.float32)
    w2 = np.asarray(w2, np.float32)
    b2 = np.asarray(b2, np.float32)
    w_fc = np.asarray(w_fc, np.float32)
    b_fc = np.asarray(b_fc, np.float32)
    w_out = np.asarray(w_out, np.float32)

    # xrep[t, ri*5+s, (b, rg, c)] = x[t, b, rg*6+ri, c+s]; row 50 = 1
    rows = (np.arange(4) * 6)[:, None] + np.arange(10)[None, :]  # [rg, ri]
    xw = x[:, :, rows, :]  # [T, B, rg4, ri10, 28]
    xrep = np.empty((T, B, 51, 4, 24), np.float32)
    for s in range(5):
        # [T, B, rg, ri, 24] -> place at k = ri*5+s
        xrep[:, :, np.arange(10) * 5 + s, :, :] = (
            xw[:, :, :, :, s:s + 24].transpose(0, 1, 3, 2, 4))
    xrep[:, :, 50] = 1.0
    # -> [ND, T, 51, (b, rg, c)]
    xrep = xrep.reshape(T, ND, BL, 51, 4, 24).transpose(1, 0, 3, 2, 4, 5)
    xrep = np.ascontiguousarray(xrep).reshape(ND, T, 51, BL * 4 * 24).astype(BF)

    # w1kn[ri*5+s, p=jq*64+jp*20+oc] = 0.1*w1[oc, ri-j, s], j=2jp+jq
    w1kn = np.zeros((51, 128), np.float32)
    for jq in range(2):
        for jp in range(3):
            j = 2 * jp + jq
            p0 = jq * 64 + jp * 20
            for ri in range(10):
                dr = ri - j
                if 0 <= dr < 5:
                    for s in range(5):
                        w1kn[ri * 5 + s, p0:p0 + 20] = 0.1 * w1[:, 0, dr, s]
            w1kn[50, p0:p0 + 20] = 0.1 * b1
    w1kn = w1kn.astype(BF)

    # w2dr[dr, dc*20+ic, oc] = w2[oc, ic, dr, dc]; row 100 = b2/5
    w2dr = np.zeros((5, 101, 50), np.float32)
    for dr in range(5):
        for dc in range(5):
            w2dr[dr, dc * 20:(dc + 1) * 20, :] = w2[:, :, dr, dc].T
        w2dr[dr, 100, :] = b2 / 5.0
    w2dr = w2dr.astype(BF)

    # wfct[k=(rc)*50+oc, n] = 0.1*w_fc[n, oc*16+rc]; rows 800.. = 0.1*b_fc/32
    wfct = np.zeros((832, 500), np.float32)
    wf = w_fc.reshape(500, 50, 16)  # [n, oc, rc]
    wfct[:800] = 0.1 * wf.transpose(2, 1, 0).reshape(800, 500)
    wfct[800:832] = 0.1 * b_fc[None, :] / 32.0
    wfct = wfct.astype(BF)

    woutt = np.zeros((512, 10), np.float32)
    woutt[:500] = 0.1 * w_out.T
    woutt = woutt.astype(BF)

    ident = np.zeros((128, 128), np.float32)
    ident[np.arange(50), np.arange(50)] = 1.0
    ident[64 + np.arange(50), np.arange(50)] = 1.0
    ident = ident.astype(BF)

    return xrep, w1kn, w2dr, wfct, woutt, ident


_NC = None


def kernel(x, w1, b1, w2, b2, w_fc, b_fc, w_out):
    global _NC
    from concourse.bass_utils import run_bass_kernel_spmd

    xrep, w1kn, w2dr, wfct, woutt, ident = host_prep(
        x, w1, b1, w2, b2, w_fc, b_fc, w_out)
    if _NC is None:
        _NC = build_nc()
    in_maps = [{
        "xrep": np.ascontiguousarray(xrep[core]),
        "w1kn": w1kn, "w2dr": w2dr, "wfct": wfct,
        "woutt": woutt, "ident": ident,
    } for core in range(ND)]
    res = run_bass_kernel_spmd(_NC, in_maps, core_ids=list(range(ND)))
    # out [32, 660] f32 per core; cols 10:650 = vo_t
    out = np.empty((T, B, 10), np.float32)
    for core in range(ND):
        o = np.asarray(res.results[core]["out"], np.float32)
        out[:, core * BL:(core + 1) * BL, :] = (
            o[:, 10:650].reshape(BL, T, 10).transpose(1, 0, 2))
    return out
